# revision 35
# baseline (speedup 1.0000x reference)
"""Polynomial flow regularizer loss on 8 Trainium2 NeuronCores.

reference semantics: fit a quadratic polynomial surface (basis
[1, x, y, x^2, x*y, y^2] over a [-1,1]^2 grid) to each (b, c) image of
flow_field (64, 2, 512, 512) via least squares, and return
mean_b(sum_c(mean_pixels((f - fit)^2))).

Math used here: with Phi the (N, 6) basis, G = Phi^T Phi and r = Phi^T f,
the residual energy is  ||f - Phi G^-1 r||^2 = ||f||^2 - r^T G^-1 r.
The basis is separable in (x, y), so r is recoverable from the 3x512
matrix V[a, w] = sum_h y_h^a f[h, w]  (a = 0, 1, 2) via
r_{(a,b)} = sum_w V[a, w] x_w^b.

Device work per image (512x512):
  - V via 4 accumulating TensorE matmuls: lhsT = y-basis chunk (128, 3),
    rhs = image h-chunk (128, 512), PSUM accumulate over the 4 chunks.
  - per-partition sum of squares, on ScalarE (Square + accum_out) for
    even images and VectorE (tensor_tensor_reduce mult/add) for odd
    images so the two engines split the elementwise pass.
Host work: the 6-vector r per image, the 6x6 solve, and the final mean —
a few thousand flops on ~100KB of device output.

Sharding: data-parallel over batch. Core k takes batches [8k, 8k+8)
= 16 images = 16MB; the loss contributions are summed on host.
"""

import sys

import numpy as np

sys.path.insert(0, "/opt/trn_rl_repo")

import concourse.bacc as bacc
import concourse.bass as bass
import concourse.tile as tile
from concourse import mybir
from concourse.bass_utils import run_bass_kernel_spmd

B, C, H, W = 64, 2, 512, 512
N_CORES = 8
IMGS = (B // N_CORES) * C  # images per core
GROUP = 2  # images per DMA/PSUM chunk
N_GROUPS = IMGS // GROUP
HCHUNKS = H // 128  # sub-rows per partition
F32 = mybir.dt.float32
DVE_MOD = 4  # images with i % DVE_MOD == DVE_MOD-1 square on VectorE

_NC = None


def _build():
    F32R = mybir.dt.float32r
    nc = bacc.Bacc()
    # float32r = same 4-byte payload; declared as f32r so the PE runs the
    # single-pass (1 cycle/row) fp32r matmul instead of the 4-cycle fp32 one
    flow = nc.declare_dram_parameter("flow", [IMGS, H, W], F32R, isOutput=False)
    # [:, :12] interleaved order (h = 4p+s), [:, 12:] chunk order (h = 128t+p)
    ybas = nc.declare_dram_parameter(
        "ybasis", [128, 6 * HCHUNKS], F32R, isOutput=False
    )
    v_out = nc.declare_dram_parameter("v_out", [3, IMGS, W], F32, isOutput=True)
    sq_out = nc.declare_dram_parameter("sq_out", [128, 2 * IMGS], F32, isOutput=True)

    with tile.TileContext(nc) as tc:
        with (
            tc.tile_pool(name="const", bufs=1) as cpool,
            tc.tile_pool(name="img", bufs=6) as ipool,
            tc.tile_pool(name="imglast", bufs=1) as ilpool,
            tc.tile_pool(name="scr", bufs=2) as spool,
            tc.tile_pool(name="psum", bufs=4, space="PSUM") as ppool,
        ):
            yb = cpool.tile([128, 6 * HCHUNKS], F32R)
            nc.scalar.dma_start(out=yb[:], in_=ybas[:])
            # only the TOTAL sum of squares matters for the loss, so the
            # square work can be split arbitrarily across engines/columns
            sq_a = cpool.tile([128, IMGS], F32)
            sq_d = cpool.tile([128, IMGS], F32)
            v_stage = cpool.tile([3, IMGS, W], F32)

            # 2MB input DMAs stream at ~400GB/s (1MB measured ~334); the
            # last 2MB goes as 1MB + 4x256KB to shorten the compute tail.
            chunks = [2] * ((IMGS - 2) // 2) + [1]
            i0 = 0
            for chunk in chunks:
                img = ipool.tile([128, chunk, HCHUNKS, W], F32R, tag="img")
                nc.sync.dma_start(
                    out=img[:],
                    in_=flow[i0 : i0 + chunk].rearrange(
                        "i (p s) w -> p i s w", p=128
                    ),
                )

                for j in range(chunk):
                    i = i0 + j
                    psum = ppool.tile([3, W], F32)
                    for s in range(HCHUNKS):
                        nc.tensor.matmul(
                            psum[:, :],
                            yb[:, 3 * s : 3 * s + 3],
                            img[:, j, s, :],
                            start=(s == 0),
                            stop=(s == HCHUNKS - 1),
                        )

                    # squares: ScalarE takes sub-rows 0..2, VectorE sub-row
                    # 3 (TENSOR_TENSOR_REDUCE crashes TRN2 here, so plain
                    # mul+reduce on DVE instead)
                    scr_a = spool.tile([128, 3, W], F32, tag="scra")
                    nc.scalar.activation(
                        out=scr_a[:],
                        in_=img[:, j, 0:3, :].bitcast(F32),
                        func=mybir.ActivationFunctionType.Square,
                        accum_out=sq_a[:, i : i + 1],
                    )
                    scr_d = spool.tile([128, W], F32, tag="scrd")
                    nc.vector.tensor_mul(
                        scr_d[:],
                        img[:, j, 3, :].bitcast(F32),
                        img[:, j, 3, :].bitcast(F32),
                    )
                    nc.vector.reduce_sum(
                        out=sq_d[:, i : i + 1],
                        in_=scr_d[:],
                        axis=mybir.AxisListType.X,
                    )

                    nc.vector.tensor_copy(out=v_stage[:, i, :], in_=psum[:])
                i0 += chunk

            # last image: 4 x 256KB row-block DMAs in chunk order
            # (h = 128t+p, contiguous per partition) so each matmul /
            # square starts as soon as its block lands
            i = IMGS - 1
            img = ilpool.tile([128, HCHUNKS, W], F32R)
            for t in range(HCHUNKS):
                nc.sync.dma_start(
                    out=img[:, t, :],
                    in_=flow[i, 128 * t : 128 * (t + 1), :],
                )
            psum = ppool.tile([3, W], F32)
            for t in range(HCHUNKS):
                nc.tensor.matmul(
                    psum[:, :],
                    yb[:, 3 * (HCHUNKS + t) : 3 * (HCHUNKS + t) + 3],
                    img[:, t, :],
                    start=(t == 0),
                    stop=(t == HCHUNKS - 1),
                )
            scr_a = spool.tile([128, 3, W], F32, tag="scra")
            nc.scalar.activation(
                out=scr_a[:],
                in_=img[:, 0:3, :].bitcast(F32),
                func=mybir.ActivationFunctionType.Square,
                accum_out=sq_a[:, i : i + 1],
            )
            scr_d = spool.tile([128, W], F32, tag="scrd")
            nc.vector.tensor_mul(
                scr_d[:], img[:, 3, :].bitcast(F32), img[:, 3, :].bitcast(F32)
            )
            nc.vector.reduce_sum(
                out=sq_d[:, i : i + 1], in_=scr_d[:], axis=mybir.AxisListType.X
            )
            nc.vector.tensor_copy(out=v_stage[:, i, :], in_=psum[:])

            nc.scalar.dma_start(out=v_out[:], in_=v_stage[:])
            nc.scalar.dma_start(out=sq_out[:, 0:IMGS], in_=sq_a[:])
            nc.scalar.dma_start(out=sq_out[:, IMGS:], in_=sq_d[:])
    nc.finalize()
    return nc


def _ybasis():
    y = np.linspace(-1.0, 1.0, H, dtype=np.float32)
    Y = np.empty((128, 6 * HCHUNKS), dtype=np.float32)
    for s in range(HCHUNKS):
        seg = y[s::HCHUNKS]  # interleaved: h = HCHUNKS*p + s
        Y[:, 3 * s + 0] = 1.0
        Y[:, 3 * s + 1] = seg
        Y[:, 3 * s + 2] = seg * seg
    for t in range(HCHUNKS):
        seg = y[128 * t : 128 * (t + 1)]  # chunked: h = 128*t + p
        Y[:, 3 * (HCHUNKS + t) + 0] = 1.0
        Y[:, 3 * (HCHUNKS + t) + 1] = seg
        Y[:, 3 * (HCHUNKS + t) + 2] = seg * seg
    return Y


def _gram():
    # G = Phi^T Phi for basis [1, x, y, x^2, x*y, y^2]; exploits
    # separability: each entry is (sum_h y^ay) * (sum_w x^ax).
    g = np.linspace(-1.0, 1.0, H, dtype=np.float32).astype(np.float64)
    pw = np.stack([np.ones_like(g), g, g * g, g**3, g**4])  # powers 0..4
    s = pw.sum(axis=1)  # s[k] = sum grid^k
    # exponents (ay, ax) per basis fn
    e = [(0, 0), (0, 1), (1, 0), (0, 2), (1, 1), (2, 0)]
    G = np.empty((6, 6))
    for j in range(6):
        for k in range(6):
            ay = e[j][0] + e[k][0]
            ax = e[j][1] + e[k][1]
            G[j, k] = s[ay] * s[ax]
    return G


def _run(shards, ybasis=None, trace=False, **kwargs):
    """shards: (8, IMGS, H, W) float32. Returns BassKernelResults."""
    global _NC
    if _NC is None:
        _NC = _build()
    if ybasis is None:
        ybasis = _ybasis()
    in_maps = [
        {"flow": np.ascontiguousarray(shards[k]), "ybasis": ybasis}
        for k in range(N_CORES)
    ]
    return run_bass_kernel_spmd(_NC, in_maps, list(range(N_CORES)), trace=trace, **kwargs)


def kernel(flow_field: np.ndarray) -> np.ndarray:
    flow = np.ascontiguousarray(np.asarray(flow_field, dtype=np.float32))
    assert flow.shape == (B, C, H, W)
    shards = flow.reshape(N_CORES, IMGS, H, W)

    res = _run(shards)

    G = _gram()
    x = np.linspace(-1.0, 1.0, W, dtype=np.float32).astype(np.float64)
    Xb = np.stack([np.ones_like(x), x, x * x], axis=1)  # (W, 3)

    Ginv = np.linalg.inv(G)
    total = 0.0
    for k in range(N_CORES):
        v = np.asarray(res.results[k]["v_out"], dtype=np.float64)  # (3, IMGS, W)
        sq = np.asarray(res.results[k]["sq_out"], dtype=np.float64)  # (128, IMGS)
        M = np.einsum("aiw,wb->iab", v, Xb)  # (IMGS, 3, 3)
        r = np.stack(
            [M[:, 0, 0], M[:, 0, 1], M[:, 1, 0], M[:, 0, 2], M[:, 1, 1], M[:, 2, 0]],
            axis=1,
        )  # (IMGS, 6)
        fit_energy = np.einsum("ij,jk,ik->i", r, Ginv, r)  # r^T G^-1 r
        total += float(sq.sum() - fit_energy.sum())

    loss = total / (H * W) / B
    return np.asarray(loss, dtype=np.float32)


# revision 38
# speedup vs baseline: 1.0463x; 1.0463x over previous
"""Polynomial flow regularizer loss on 8 Trainium2 NeuronCores.

reference semantics: fit a quadratic polynomial surface (basis
[1, x, y, x^2, x*y, y^2] over a [-1,1]^2 grid) to each (b, c) image of
flow_field (64, 2, 512, 512) via least squares, and return
mean_b(sum_c(mean_pixels((f - fit)^2))).

Math used here: with Phi the (N, 6) basis, G = Phi^T Phi and r = Phi^T f,
the residual energy is  ||f - Phi G^-1 r||^2 = ||f||^2 - r^T G^-1 r.
The basis is separable in (x, y), so r is recoverable from the 3x512
matrix V[a, w] = sum_h y_h^a f[h, w]  (a = 0, 1, 2) via
r_{(a,b)} = sum_w V[a, w] x_w^b.

Device work per image (512x512):
  - V via 4 accumulating TensorE matmuls: lhsT = y-basis chunk (128, 3),
    rhs = image h-chunk (128, 512), PSUM accumulate over the 4 chunks.
  - per-partition sum of squares, on ScalarE (Square + accum_out) for
    even images and VectorE (tensor_tensor_reduce mult/add) for odd
    images so the two engines split the elementwise pass.
Host work: the 6-vector r per image, the 6x6 solve, and the final mean —
a few thousand flops on ~100KB of device output.

Sharding: data-parallel over batch. Core k takes batches [8k, 8k+8)
= 16 images = 16MB; the loss contributions are summed on host.
"""

import sys

import numpy as np

sys.path.insert(0, "/opt/trn_rl_repo")

import concourse.bacc as bacc
import concourse.bass as bass
import concourse.tile as tile
from concourse import mybir
from concourse.bass_utils import run_bass_kernel_spmd

B, C, H, W = 64, 2, 512, 512
N_CORES = 8
IMGS = (B // N_CORES) * C  # images per core
GROUP = 2  # images per DMA/PSUM chunk
N_GROUPS = IMGS // GROUP
HCHUNKS = H // 128  # sub-rows per partition
F32 = mybir.dt.float32
DVE_MOD = 4  # images with i % DVE_MOD == DVE_MOD-1 square on VectorE

_NC = None


def _build():
    F32R = mybir.dt.float32r
    nc = bacc.Bacc()
    # float32r = same 4-byte payload; declared as f32r so the PE runs the
    # single-pass (1 cycle/row) fp32r matmul instead of the 4-cycle fp32 one
    flow = nc.declare_dram_parameter("flow", [IMGS, H, W], F32R, isOutput=False)
    # [:, :12] interleaved order (h = 4p+s), [:, 12:] chunk order (h = 128t+p)
    ybas = nc.declare_dram_parameter(
        "ybasis", [128, 6 * HCHUNKS], F32R, isOutput=False
    )
    v_out = nc.declare_dram_parameter("v_out", [3, IMGS, W], F32, isOutput=True)
    sq_out = nc.declare_dram_parameter("sq_out", [128, 2 * IMGS], F32, isOutput=True)

    with tile.TileContext(nc) as tc:
        with (
            tc.tile_pool(name="const", bufs=1) as cpool,
            tc.tile_pool(name="img", bufs=6) as ipool,
            tc.tile_pool(name="imglast", bufs=1) as ilpool,
            tc.tile_pool(name="scr", bufs=2) as spool,
            tc.tile_pool(name="psum", bufs=4, space="PSUM") as ppool,
        ):
            yb = cpool.tile([128, 6 * HCHUNKS], F32R)
            nc.scalar.dma_start(out=yb[:], in_=ybas[:])
            # only the TOTAL sum of squares matters for the loss, so the
            # square work can be split arbitrarily across engines/columns
            sq_a = cpool.tile([128, IMGS], F32)
            sq_d = cpool.tile([128, IMGS], F32)
            v_stage = cpool.tile([3, IMGS, W], F32)

            # 2MB input DMAs stream at ~400GB/s (1MB measured ~334); the
            # last 2MB goes as 1MB + 4x256KB to shorten the compute tail.
            chunks = [2] * ((IMGS - 2) // 2) + [1]
            i0 = 0
            for chunk in chunks:
                img = ipool.tile([128, chunk, HCHUNKS, W], F32R, tag="img")
                nc.sync.dma_start(
                    out=img[:],
                    in_=flow[i0 : i0 + chunk].rearrange(
                        "i (p s) w -> p i s w", p=128
                    ),
                )

                for j in range(chunk):
                    i = i0 + j
                    psum = ppool.tile([3, W], F32)
                    for s in range(HCHUNKS):
                        nc.tensor.matmul(
                            psum[:, :],
                            yb[:, 3 * s : 3 * s + 3],
                            img[:, j, s, :],
                            start=(s == 0),
                            stop=(s == HCHUNKS - 1),
                        )

                    # squares: ScalarE takes sub-rows 0..2, VectorE sub-row
                    # 3 (TENSOR_TENSOR_REDUCE crashes TRN2 here, so plain
                    # mul+reduce on DVE instead)
                    scr_a = spool.tile([128, 3, W], F32, tag="scra")
                    nc.scalar.activation(
                        out=scr_a[:],
                        in_=img[:, j, 0:3, :].bitcast(F32),
                        func=mybir.ActivationFunctionType.Square,
                        accum_out=sq_a[:, i : i + 1],
                    )
                    scr_d = spool.tile([128, W], F32, tag="scrd")
                    nc.vector.tensor_mul(
                        scr_d[:],
                        img[:, j, 3, :].bitcast(F32),
                        img[:, j, 3, :].bitcast(F32),
                    )
                    nc.vector.reduce_sum(
                        out=sq_d[:, i : i + 1],
                        in_=scr_d[:],
                        axis=mybir.AxisListType.X,
                    )

                    nc.vector.tensor_copy(out=v_stage[:, i, :], in_=psum[:])
                    if i == 11:
                        # flush the finished part of V mid-stream; only the
                        # last 4 images' V rides the kernel tail
                        nc.scalar.dma_start(
                            out=v_out[:, 0:12, :], in_=v_stage[:, 0:12, :]
                        )
                i0 += chunk

            # last image: 4 x 256KB row-block DMAs in chunk order
            # (h = 128t+p, contiguous per partition) so each matmul /
            # square starts as soon as its block lands
            i = IMGS - 1
            img = ilpool.tile([128, HCHUNKS, W], F32R)
            for t in range(HCHUNKS):
                nc.sync.dma_start(
                    out=img[:, t, :],
                    in_=flow[i, 128 * t : 128 * (t + 1), :],
                )
            psum = ppool.tile([3, W], F32)
            for t in range(HCHUNKS):
                nc.tensor.matmul(
                    psum[:, :],
                    yb[:, 3 * (HCHUNKS + t) : 3 * (HCHUNKS + t) + 3],
                    img[:, t, :],
                    start=(t == 0),
                    stop=(t == HCHUNKS - 1),
                )
            scr_a = spool.tile([128, 3, W], F32, tag="scra")
            nc.scalar.activation(
                out=scr_a[:],
                in_=img[:, 0:3, :].bitcast(F32),
                func=mybir.ActivationFunctionType.Square,
                accum_out=sq_a[:, i : i + 1],
            )
            scr_d = spool.tile([128, W], F32, tag="scrd")
            nc.vector.tensor_mul(
                scr_d[:], img[:, 3, :].bitcast(F32), img[:, 3, :].bitcast(F32)
            )
            nc.vector.reduce_sum(
                out=sq_d[:, i : i + 1], in_=scr_d[:], axis=mybir.AxisListType.X
            )
            nc.vector.tensor_copy(out=v_stage[:, i, :], in_=psum[:])

            # three engines issue the outputs in parallel so the tail pays
            # one issue+receipt latency, not three back-to-back
            nc.scalar.dma_start(out=v_out[:, 12:, :], in_=v_stage[:, 12:, :])
            nc.sync.dma_start(out=sq_out[:, 0:IMGS], in_=sq_a[:])
            nc.gpsimd.dma_start(out=sq_out[:, IMGS:], in_=sq_d[:])
    nc.finalize()
    return nc


def _ybasis():
    y = np.linspace(-1.0, 1.0, H, dtype=np.float32)
    Y = np.empty((128, 6 * HCHUNKS), dtype=np.float32)
    for s in range(HCHUNKS):
        seg = y[s::HCHUNKS]  # interleaved: h = HCHUNKS*p + s
        Y[:, 3 * s + 0] = 1.0
        Y[:, 3 * s + 1] = seg
        Y[:, 3 * s + 2] = seg * seg
    for t in range(HCHUNKS):
        seg = y[128 * t : 128 * (t + 1)]  # chunked: h = 128*t + p
        Y[:, 3 * (HCHUNKS + t) + 0] = 1.0
        Y[:, 3 * (HCHUNKS + t) + 1] = seg
        Y[:, 3 * (HCHUNKS + t) + 2] = seg * seg
    return Y


def _gram():
    # G = Phi^T Phi for basis [1, x, y, x^2, x*y, y^2]; exploits
    # separability: each entry is (sum_h y^ay) * (sum_w x^ax).
    g = np.linspace(-1.0, 1.0, H, dtype=np.float32).astype(np.float64)
    pw = np.stack([np.ones_like(g), g, g * g, g**3, g**4])  # powers 0..4
    s = pw.sum(axis=1)  # s[k] = sum grid^k
    # exponents (ay, ax) per basis fn
    e = [(0, 0), (0, 1), (1, 0), (0, 2), (1, 1), (2, 0)]
    G = np.empty((6, 6))
    for j in range(6):
        for k in range(6):
            ay = e[j][0] + e[k][0]
            ax = e[j][1] + e[k][1]
            G[j, k] = s[ay] * s[ax]
    return G


def _run(shards, ybasis=None, trace=False, **kwargs):
    """shards: (8, IMGS, H, W) float32. Returns BassKernelResults."""
    global _NC
    if _NC is None:
        _NC = _build()
    if ybasis is None:
        ybasis = _ybasis()
    in_maps = [
        {"flow": np.ascontiguousarray(shards[k]), "ybasis": ybasis}
        for k in range(N_CORES)
    ]
    return run_bass_kernel_spmd(_NC, in_maps, list(range(N_CORES)), trace=trace, **kwargs)


def kernel(flow_field: np.ndarray) -> np.ndarray:
    flow = np.ascontiguousarray(np.asarray(flow_field, dtype=np.float32))
    assert flow.shape == (B, C, H, W)
    shards = flow.reshape(N_CORES, IMGS, H, W)

    res = _run(shards)

    G = _gram()
    x = np.linspace(-1.0, 1.0, W, dtype=np.float32).astype(np.float64)
    Xb = np.stack([np.ones_like(x), x, x * x], axis=1)  # (W, 3)

    Ginv = np.linalg.inv(G)
    total = 0.0
    for k in range(N_CORES):
        v = np.asarray(res.results[k]["v_out"], dtype=np.float64)  # (3, IMGS, W)
        sq = np.asarray(res.results[k]["sq_out"], dtype=np.float64)  # (128, IMGS)
        M = np.einsum("aiw,wb->iab", v, Xb)  # (IMGS, 3, 3)
        r = np.stack(
            [M[:, 0, 0], M[:, 0, 1], M[:, 1, 0], M[:, 0, 2], M[:, 1, 1], M[:, 2, 0]],
            axis=1,
        )  # (IMGS, 6)
        fit_energy = np.einsum("ij,jk,ik->i", r, Ginv, r)  # r^T G^-1 r
        total += float(sq.sum() - fit_energy.sum())

    loss = total / (H * W) / B
    return np.asarray(loss, dtype=np.float32)
